# revision 15
# baseline (speedup 1.0000x reference)
"""GAT layer (nn_GATLayer) Trainium2 Bass kernel.

Reference math:
    NF = x @ W.T + b                     # [N, 256] -> heads [N, 8, 32]
    lp[i,h] = sum_d NF[i,h,d] * a[h,d];  lc[j,h] = sum_d NF[j,h,d] * a[h,32+d]
    logits[i,j,h] = leaky_relu(lp+lc, 0.2) masked to 0 where adj==0
    out[i,h,:] = softmax_j(logits) @ NF[:,h,:]

Identities used (adj in {0,1}):
    exp(leaky_relu(z, .2)) = exp(.2 z) * max(exp(.8 z), 1)
    em[i,j,h] = exp(masked logits) = 1 - adj + adj*A2[i]*B2[j]*max(A8[i]*B8[j], 1)
      where A8=exp(.8 lp), B8=exp(.8 lc), A2=exp(.2 lp), B2=exp(.2 lc)
    u'[j,i,h] = adj[i,j] * max(A8[i]*B8[j], 1)        # ONE masked stream per head
    num[i,h,c] = S[h,c] - M3[i,c] + A2[i,h] * (u' @ (B2*NF)_h)[i,c]
    Z[i,h]     = N - deg[i]      + A2[i,h] * (u' @ B2_h)[i]
    out = num / Z
    with S = colsum(NF), M3 = adj @ NF, deg = adj @ 1.

Per core (rows sharded, 512 each): j-loop over 32 chunks of 128, two rounds
to fit PSUM banks: round A = shared(M3) psums + heads 0-5, round B = deg +
heads 6-7. u' built as: TS (a8m = (A8rep * B8[j]) max 1) + TT (a8m * adjT),
bf16, with some head-tiles offloaded to ACT (2-op max via relu) and TT
slices to GPSIMD for engine balance.
"""

import numpy as np
import ml_dtypes

import concourse.bass as bass
import concourse.bacc as bacc
import concourse.tile as tile
from concourse import mybir
from concourse.bass_utils import run_bass_kernel_spmd
from concourse.masks import make_identity

N_CORES = 8
N = 4096
IN_FEAT = 256
OUT_FEAT = 256
H = 8
D = 32
R = N // N_CORES          # rows (parents) per core = 512
JC = N // 128             # j-chunks of 128 = 32
KA = IN_FEAT + 1          # augmented contraction (bias row) = 257
WCOLS = D + 1             # per-head weight cols (B2NF slice + B2 col) = 33

FP = mybir.dt.float32
BF = mybir.dt.bfloat16

ROUND_A = [0, 1, 2, 3, 4, 5]
ROUND_B = [6, 7]
# engine assignment for the a8m op per head: 'dve' (TS) or 'act' (2 ACT ops)
A8M_ENGINE = {0: 'dve', 1: 'dve', 2: 'dve', 3: 'dve', 4: 'dve', 5: 'dve',
              6: 'dve', 7: 'act'}
# TT (mask multiply) engine per head
TT_ENGINE = {0: 'dve', 1: 'dve', 2: 'dve', 3: 'dve', 4: 'gp', 5: 'gp',
             6: 'gp', 7: 'dve'}


def build_program():
    nc = bacc.Bacc("TRN2", target_bir_lowering=False, debug=False,
                   num_devices=N_CORES)

    xTa = nc.dram_tensor("xTa", [KA, N], FP, kind="ExternalInput").ap()
    xTrows = nc.dram_tensor("xTrows", [KA, R], FP, kind="ExternalInput").ap()
    wTa = nc.dram_tensor("wTa", [KA, OUT_FEAT], FP, kind="ExternalInput").ap()
    WAc = nc.dram_tensor("WAc", [KA, H], FP, kind="ExternalInput").ap()
    WAp = nc.dram_tensor("WAp", [KA, H], FP, kind="ExternalInput").ap()
    adjT = nc.dram_tensor("adjT", [N, R], BF, kind="ExternalInput").ap()
    sel8_in = nc.dram_tensor("sel8", [H, H * 128], FP, kind="ExternalInput").ap()
    outT = nc.dram_tensor("outT", [OUT_FEAT, R], FP, kind="ExternalOutput").ap()

    with tile.TileContext(nc) as tc:
        from contextlib import ExitStack
        with ExitStack() as top:
            consts = top.enter_context(tc.tile_pool(name="consts", bufs=1))
            persist = top.enter_context(tc.tile_pool(name="persist", bufs=1))

            ident = consts.tile([128, 128], FP)
            make_identity(nc, ident[:])
            ones_col = consts.tile([128, 1], BF)
            nc.vector.memset(ones_col[:], 1.0)
            sel8 = consts.tile([H, H * 128], FP)
            nc.sync.dma_start(out=sel8[:], in_=sel8_in[:])
            negone = consts.tile([128, 1], FP)
            nc.vector.memset(negone[:], -1.0)

            wk = consts.tile([128, 2, OUT_FEAT], FP)
            nc.sync.dma_start(out=wk[:, 0, :], in_=wTa[0:128, :])
            nc.sync.dma_start(out=wk[:, 1, :], in_=wTa[128:256, :])
            wk2 = consts.tile([1, OUT_FEAT], FP)
            nc.sync.dma_start(out=wk2[:], in_=wTa[256:257, :])
            wac = consts.tile([128, 2, H], FP)
            nc.sync.dma_start(out=wac[:, 0, :], in_=WAc[0:128, :])
            nc.sync.dma_start(out=wac[:, 1, :], in_=WAc[128:256, :])
            wac2 = consts.tile([1, H], FP)
            nc.sync.dma_start(out=wac2[:], in_=WAc[256:257, :])
            wap = consts.tile([128, 2, H], FP)
            nc.sync.dma_start(out=wap[:, 0, :], in_=WAp[0:128, :])
            nc.sync.dma_start(out=wap[:, 1, :], in_=WAp[128:256, :])
            wap2 = consts.tile([1, H], FP)
            nc.sync.dma_start(out=wap2[:], in_=WAp[256:257, :])

            # Persistent SBUF
            aggW = persist.tile([128, JC, H * WCOLS], BF)   # [B2NF_h | B2_h] x8
            shW = persist.tile([128, JC, OUT_FEAT + 1], BF)  # [NF | ones]
            lcn = persist.tile([128, JC, H], FP)
            b8c = persist.tile([128, JC, H], FP)             # exp(.8 lc)
            lpT = persist.tile([H, R], FP)
            a8rep = persist.tile([128, H, R], FP)            # exp(.8 lp) bcast
            a2rep = persist.tile([128, H, R], FP)            # exp(.2 lp) bcast
            scol = persist.tile([128, 2], FP)
            numT = persist.tile([128, 2, R], FP)
            outTs = persist.tile([128, 2, R], FP)
            zAll32 = persist.tile([32, R], FP)
            rzAll = persist.tile([32, R], FP)
            m3s = persist.tile([128, 2, R], FP)
            zrow6 = persist.tile([1, H, R], FP)
            degs = persist.tile([1, R], FP)
            nc.vector.memset(zAll32[:], 1.0)

            aggW_v = aggW.rearrange("p j (h w) -> p j h w", w=WCOLS)
            nc.vector.memset(shW[:, :, OUT_FEAT:OUT_FEAT + 1], 1.0)

            # ---- Phase 0: NF, lc, lp, exp factors, S ----
            with ExitStack() as ph0:
                xw = ph0.enter_context(tc.tile_pool(name="xw", bufs=3))
                ps0 = ph0.enter_context(
                    tc.tile_pool(name="ps0", bufs=4, space="PSUM"))
                for nb in range(JC):
                    xk = xw.tile([128, 2, 128], FP)
                    nc.sync.dma_start(out=xk[:, 0, :],
                                      in_=xTa[0:128, nb * 128:(nb + 1) * 128])
                    nc.sync.dma_start(out=xk[:, 1, :],
                                      in_=xTa[128:256, nb * 128:(nb + 1) * 128])
                    xk2 = xw.tile([1, 128], FP)
                    nc.sync.dma_start(out=xk2[:],
                                      in_=xTa[256:257, nb * 128:(nb + 1) * 128])
                    pnf = ps0.tile([128, OUT_FEAT], FP, space="PSUM", tag="ps0")
                    plc = ps0.tile([128, H], FP, space="PSUM", tag="ps0")
                    nc.tensor.matmul(pnf[:], xk[:, 0, :], wk[:, 0, :],
                                     start=True, stop=False)
                    nc.tensor.matmul(pnf[:], xk[:, 1, :], wk[:, 1, :],
                                     start=False, stop=False)
                    nc.tensor.matmul(pnf[:], xk2[:], wk2[:],
                                     start=False, stop=True)
                    nc.tensor.matmul(plc[:], xk[:, 0, :], wac[:, 0, :],
                                     start=True, stop=False)
                    nc.tensor.matmul(plc[:], xk[:, 1, :], wac[:, 1, :],
                                     start=False, stop=False)
                    nc.tensor.matmul(plc[:], xk2[:], wac2[:],
                                     start=False, stop=True)
                    # NF (bf16) into shared weights
                    nc.scalar.copy(shW[:, nb, 0:OUT_FEAT], pnf[:])
                    nc.scalar.copy(lcn[:, nb, :], plc[:])

                # exp factors of lc: B8 = exp(.8 lc) (scalar APs), B2 = exp(.2 lc)
                lcn_f = lcn.rearrange("p j h -> p (j h)")
                b8c_f = b8c.rearrange("p j h -> p (j h)")
                nc.scalar.activation(b8c_f, lcn_f,
                                     mybir.ActivationFunctionType.Exp,
                                     bias=0.0, scale=0.8)
                b2tmp = xw.tile([128, JC, H], FP, name="b2tmp")
                nc.scalar.activation(b2tmp.rearrange("p j h -> p (j h)"), lcn_f,
                                     mybir.ActivationFunctionType.Exp,
                                     bias=0.0, scale=0.2)
                # aggW: B2NF = NF * B2 (broadcast B2 over the 32 feature cols)
                for nb in range(JC):
                    b2b = b2tmp[:, nb, :]
                    b2bc = bass.AP(tensor=b2b.tensor, offset=b2b.offset,
                                   ap=[b2b.ap[0], b2b.ap[1], [0, D]])
                    nfv = shW[:, nb, 0:OUT_FEAT].rearrange(
                        "p (h d) -> p h d", d=D)
                    nc.gpsimd.tensor_mul(aggW_v[:, nb, :, 0:D], nfv, b2bc)
                    b2col = b2tmp[:, nb, :]
                    b2col3 = bass.AP(tensor=b2col.tensor, offset=b2col.offset,
                                     ap=[b2col.ap[0], b2col.ap[1], [0, 1]])
                    nc.gpsimd.tensor_copy(aggW_v[:, nb, :, D:D + 1], b2col3)

                # lp for this core's own rows
                for rb in range(R // 128):
                    xr = xw.tile([128, 2, 128], FP)
                    nc.sync.dma_start(out=xr[:, 0, :],
                                      in_=xTrows[0:128, rb * 128:(rb + 1) * 128])
                    nc.sync.dma_start(out=xr[:, 1, :],
                                      in_=xTrows[128:256, rb * 128:(rb + 1) * 128])
                    xr2 = xw.tile([1, 128], FP)
                    nc.sync.dma_start(
                        out=xr2[:], in_=xTrows[256:257, rb * 128:(rb + 1) * 128])
                    plp = ps0.tile([128, H], FP, space="PSUM", tag="ps0")
                    nc.tensor.matmul(plp[:], xr[:, 0, :], wap[:, 0, :],
                                     start=True, stop=False)
                    nc.tensor.matmul(plp[:], xr[:, 1, :], wap[:, 1, :],
                                     start=False, stop=False)
                    nc.tensor.matmul(plp[:], xr2[:], wap2[:],
                                     start=False, stop=True)
                    lps = xw.tile([128, H], FP)
                    nc.scalar.copy(lps[:], plp[:])
                    plpT = ps0.tile([H, 128], FP, space="PSUM", tag="ps0")
                    nc.tensor.transpose(plpT[:], lps[:], ident[:])
                    nc.scalar.copy(lpT[:, rb * 128:(rb + 1) * 128], plpT[:])

                # S = colsum(NF) from bf16 weights (matches aggregation dtype)
                psS = ps0.tile([1, OUT_FEAT], FP, space="PSUM", tag="ps0")
                for nb in range(JC):
                    nc.tensor.matmul(psS[:], ones_col[:], shW[:, nb, 0:OUT_FEAT],
                                     start=(nb == 0), stop=(nb == JC - 1))
                sS = xw.tile([1, OUT_FEAT], FP)
                nc.scalar.copy(sS[:], psS[:])
                for h in range(H):
                    pst = ps0.tile([D, 1], FP, space="PSUM", tag="ps0")
                    nc.tensor.transpose(
                        pst[:], sS[0:1, h * D:(h + 1) * D], ident[0:1, 0:1])
                    r0 = (h * D) % 128
                    nc.scalar.copy(scol[r0:r0 + D, h // 4:h // 4 + 1], pst[:])

                # A8/A2 row factors, broadcast across partitions
                a8T = xw.tile([H, R], FP)
                nc.scalar.activation(a8T[:], lpT[:],
                                     mybir.ActivationFunctionType.Exp,
                                     bias=0.0, scale=0.8)
                a2T = xw.tile([H, R], FP)
                nc.scalar.activation(a2T[:], lpT[:],
                                     mybir.ActivationFunctionType.Exp,
                                     bias=0.0, scale=0.2)
                for h in range(H):
                    pbr = ps0.tile([128, R], FP, space="PSUM", tag="psbig")
                    nc.tensor.matmul(pbr[:], sel8[:, h * 128:(h + 1) * 128],
                                     a8T[:], start=True, stop=True)
                    nc.scalar.copy(a8rep[:, h, :], pbr[:])
                    pbr2 = ps0.tile([128, R], FP, space="PSUM", tag="psbig")
                    nc.tensor.matmul(pbr2[:], sel8[:, h * 128:(h + 1) * 128],
                                     a2T[:], start=True, stop=True)
                    nc.scalar.copy(a2rep[:, h, :], pbr2[:])

            # ---- Phase 1: main j-loop, two rounds ----
            def do_round(ph, heads, with_m3, with_deg):
                acc = ph.enter_context(
                    tc.tile_pool(name="acc", bufs=1, space="PSUM"))
                stream = ph.enter_context(tc.tile_pool(name="stream", bufs=3))
                work = ph.enter_context(tc.tile_pool(name="work", bufs=3))
                pacc = {h: acc.tile([WCOLS, R], FP, space="PSUM",
                                    name=f"pacc{h}") for h in heads}
                psh = None
                pdeg = None
                if with_m3:
                    psh = [acc.tile([128, R], FP, space="PSUM", name=f"psh{k}")
                           for k in range(2)]
                if with_deg:
                    pdeg = acc.tile([1, R], FP, space="PSUM", name="pdeg")
                nh = len(heads)
                for jc in range(JC):
                    at = stream.tile([128, R], BF, name="adjT_tile")
                    nc.sync.dma_start(out=at[:],
                                      in_=adjT[jc * 128:(jc + 1) * 128, :])
                    tb = work.tile([128, nh, R], BF, name="tb")
                    s1 = work.tile([128, nh, R], BF, name="s1")
                    for k, h in enumerate(heads):
                        if A8M_ENGINE[h] == 'act':
                            # relu(a8*B8 - 1) then +1 == max(a8*B8, 1)
                            rr = work.tile([128, R], BF, name="rr")
                            nc.scalar.activation(
                                rr[:], a8rep[:, h, :],
                                mybir.ActivationFunctionType.Relu,
                                bias=negone[:], scale=b8c[:, jc, h:h + 1])
                            nc.scalar.activation(
                                tb[:, k, :], rr[:],
                                mybir.ActivationFunctionType.Copy,
                                bias=1.0, scale=1.0)
                        else:
                            nc.vector.tensor_scalar(
                                tb[:, k, :], a8rep[:, h, :],
                                b8c[:, jc, h:h + 1], 1.0,
                                mybir.AluOpType.mult, mybir.AluOpType.max)
                    # mask multiply: fused across heads per engine
                    dve_ks = [k for k, h in enumerate(heads)
                              if TT_ENGINE[h] == 'dve']
                    gp_ks = [k for k, h in enumerate(heads)
                             if TT_ENGINE[h] == 'gp']
                    for eng, ks in ((nc.vector, dve_ks), (nc.gpsimd, gp_ks)):
                        for k0, k1 in _runs(ks):
                            cnt = k1 - k0
                            atb = bass.AP(tensor=at.tensor, offset=at.offset,
                                          ap=[at.ap[0], [0, cnt], at.ap[1]])
                            eng.tensor_mul(s1[:, k0:k1, :], tb[:, k0:k1, :],
                                           atb)
                    for k, h in enumerate(heads):
                        nc.tensor.matmul(
                            pacc[h][:],
                            aggW[:, jc, h * WCOLS:(h + 1) * WCOLS],
                            s1[:, k, :],
                            start=(jc == 0), stop=(jc == JC - 1))
                    if with_m3:
                        nc.tensor.matmul(psh[0][:], shW[:, jc, 0:128], at[:],
                                         start=(jc == 0), stop=(jc == JC - 1))
                        nc.tensor.matmul(psh[1][:], shW[:, jc, 128:256], at[:],
                                         start=(jc == 0), stop=(jc == JC - 1))
                    if with_deg:
                        nc.tensor.matmul(
                            pdeg[:], shW[:, jc, OUT_FEAT:OUT_FEAT + 1], at[:],
                            start=(jc == 0), stop=(jc == JC - 1))
                return pacc, psh, pdeg, work

            with ExitStack() as ph1:
                paccA, psh, _, workA = do_round(ph1, ROUND_A, True, False)
                for h in ROUND_A:
                    _head_epilogue(nc, h, paccA[h], psh[h // 4][
                        (h * D) % 128:(h * D) % 128 + D, :], a2rep, scol,
                        numT, zrow6, workA)
                # save M3 to SBUF before psum banks are released
                nc.scalar.copy(m3s[:, 0, :], psh[0][:])
                nc.scalar.copy(m3s[:, 1, :], psh[1][:])
            with ExitStack() as ph1b:
                paccB, _, pdeg, workB = do_round(ph1b, ROUND_B, False, True)
                for h in ROUND_B:
                    _head_epilogue(nc, h, paccB[h], m3s[
                        (h * D) % 128:(h * D) % 128 + D, h // 4, :], a2rep,
                        scol, numT, zrow6, workB)
                nc.scalar.copy(degs[:], pdeg[:])
                # finalize Z rows: z = zpart + N - deg, packed to zAll32
                for h in range(H):
                    ztmp = workB.tile([1, R], FP, name="ztmp")
                    nc.vector.scalar_tensor_tensor(
                        ztmp[:], zrow6[0:1, h, :], float(N), degs[:],
                        mybir.AluOpType.add, mybir.AluOpType.subtract)
                    nc.sync.dma_start(out=zAll32[h:h + 1, :], in_=ztmp[:])

            nc.vector.reciprocal(rzAll[:], zAll32[:])

            with ExitStack() as ph2:
                ps2 = ph2.enter_context(
                    tc.tile_pool(name="ps2", bufs=2, space="PSUM"))
                for h in range(H):
                    r0 = (h * D) % 128
                    ch = h // 4
                    pz = ps2.tile([D, R], FP, space="PSUM")
                    nc.tensor.matmul(pz[:], sel8[:, h * 128:h * 128 + D],
                                     rzAll[0:H, :], start=True, stop=True)
                    nc.vector.tensor_mul(
                        outTs[r0:r0 + D, ch, :], numT[r0:r0 + D, ch, :], pz[:])
                nc.sync.dma_start(out=outT[0:128, :], in_=outTs[:, 0, :])
                nc.sync.dma_start(out=outT[128:256, :], in_=outTs[:, 1, :])

    nc.compile()
    return nc


def _runs(ks):
    """Contiguous runs [k0, k1) in a sorted index list."""
    out = []
    for k in ks:
        if out and out[-1][1] == k:
            out[-1][1] = k + 1
        else:
            out.append([k, k + 1])
    return [tuple(x) for x in out]


def _head_epilogue(nc, h, pacc, m3, a2rep, scol, numT, zrow6, work):
    """numT_h = S[c] + A2[i]*(u'@B2NF) - M3[c,i]; zrow6_h = A2*(u'@B2)."""
    r0 = (h * D) % 128
    ch = h // 4
    t1 = work.tile([128, R], mybir.dt.float32, name="t1")
    nc.vector.tensor_mul(t1[r0:r0 + D, :], pacc[0:D, :],
                         a2rep[r0:r0 + D, h, :])
    nc.vector.scalar_tensor_tensor(
        numT[r0:r0 + D, ch, :], t1[r0:r0 + D, :],
        scol[r0:r0 + D, ch:ch + 1], m3,
        mybir.AluOpType.add, mybir.AluOpType.subtract)
    nc.vector.tensor_mul(zrow6[0:1, h, :], pacc[D:D + 1, :],
                         a2rep[32:33, h, :])


_PROGRAM_CACHE = {}


def kernel(x, W, b, a, adj_matrix):
    x = np.asarray(x, dtype=np.float32)
    W = np.asarray(W, dtype=np.float32)
    b = np.asarray(b, dtype=np.float32)
    a = np.asarray(a, dtype=np.float32)
    adj = np.asarray(adj_matrix, dtype=np.float32)

    xTa = np.ascontiguousarray(
        np.vstack([x.T, np.ones((1, N), np.float32)]))            # [257, N]
    wTa = np.ascontiguousarray(np.vstack([W.T, b[None, :]]))      # [257, 256]
    Ap = np.zeros((OUT_FEAT, H), np.float32)
    Ac = np.zeros((OUT_FEAT, H), np.float32)
    for h in range(H):
        Ap[h * D:(h + 1) * D, h] = a[h, :D]
        Ac[h * D:(h + 1) * D, h] = a[h, D:]
    WAp = np.ascontiguousarray(wTa @ Ap)
    WAc = np.ascontiguousarray(wTa @ Ac)

    sel8_host = np.zeros((H, H * 128), np.float32)
    for h in range(H):
        sel8_host[h, h * 128:(h + 1) * 128] = 1.0

    if "nc" not in _PROGRAM_CACHE:
        _PROGRAM_CACHE["nc"] = build_program()
    nc = _PROGRAM_CACHE["nc"]

    in_maps = []
    for c in range(N_CORES):
        rows = slice(c * R, (c + 1) * R)
        in_maps.append({
            "xTa": xTa,
            "xTrows": np.ascontiguousarray(xTa[:, rows]),
            "wTa": wTa,
            "WAc": WAc,
            "WAp": WAp,
            "adjT": np.ascontiguousarray(adj[rows, :].T).astype(
                ml_dtypes.bfloat16),
            "sel8": sel8_host,
        })

    res = run_bass_kernel_spmd(nc, in_maps, list(range(N_CORES)))
    out = np.empty((N, OUT_FEAT), np.float32)
    for c in range(N_CORES):
        out[c * R:(c + 1) * R, :] = res.results[c]["outT"].T
    return out


# revision 16
# speedup vs baseline: 1.1039x; 1.1039x over previous
"""GAT layer (nn_GATLayer) Trainium2 Bass kernel.

Reference math:
    NF = x @ W.T + b                     # [N, 256] -> heads [N, 8, 32]
    lp[i,h] = sum_d NF[i,h,d] * a[h,d];  lc[j,h] = sum_d NF[j,h,d] * a[h,32+d]
    logits[i,j,h] = leaky_relu(lp+lc, 0.2) masked to 0 where adj==0
    out[i,h,:] = softmax_j(logits) @ NF[:,h,:]

Identities used (adj in {0,1}):
    exp(leaky_relu(z, .2)) = exp(.2 z) * max(exp(.8 z), 1)
    em[i,j,h] = exp(masked logits) = 1 - adj + adj*A2[i]*B2[j]*max(A8[i]*B8[j], 1)
      where A8=exp(.8 lp), B8=exp(.8 lc), A2=exp(.2 lp), B2=exp(.2 lc)
    u'[j,i,h] = adj[i,j] * max(A8[i]*B8[j], 1)        # ONE masked stream per head
    num[i,h,c] = S[h,c] - M3[i,c] + A2[i,h] * (u' @ (B2*NF)_h)[i,c]
    Z[i,h]     = N - deg[i]      + A2[i,h] * (u' @ B2_h)[i]
    out = num / Z
    with S = colsum(NF), M3 = adj @ NF, deg = adj @ 1.

Per core (rows sharded, 512 each): j-loop over 32 chunks of 128, two rounds
to fit PSUM banks: round A = shared(M3) psums + heads 0-5, round B = deg +
heads 6-7. u' built as: TS (a8m = (A8rep * B8[j]) max 1) + TT (a8m * adjT),
bf16, with some head-tiles offloaded to ACT (2-op max via relu) and TT
slices to GPSIMD for engine balance.
"""

import numpy as np
import ml_dtypes

import concourse.bass as bass
import concourse.bacc as bacc
import concourse.tile as tile
from concourse import mybir
from concourse.bass_utils import run_bass_kernel_spmd
from concourse.masks import make_identity

N_CORES = 8
N = 4096
IN_FEAT = 256
OUT_FEAT = 256
H = 8
D = 32
R = N // N_CORES          # rows (parents) per core = 512
JC = N // 128             # j-chunks of 128 = 32
KA = IN_FEAT + 1          # augmented contraction (bias row) = 257
WCOLS = D + 1             # per-head weight cols (B2NF slice + B2 col) = 33

FP = mybir.dt.float32
BF = mybir.dt.bfloat16

ROUND_A = [0, 1, 2, 3, 4, 7]
ROUND_B = [5, 6]
# engine assignment for the a8m op per head: 'dve' (TS) or 'act' (2 ACT ops)
A8M_ENGINE = {0: 'dve', 1: 'dve', 2: 'dve', 3: 'dve', 4: 'dve', 5: 'dve',
              6: 'dve', 7: 'act'}
# TT (mask multiply) engine per head
TT_ENGINE = {0: 'dve', 1: 'dve', 2: 'dve', 3: 'dve', 4: 'gp', 5: 'gp',
             6: 'dve', 7: 'gp'}


def build_program():
    nc = bacc.Bacc("TRN2", target_bir_lowering=False, debug=False,
                   num_devices=N_CORES)

    xTa = nc.dram_tensor("xTa", [KA, N], FP, kind="ExternalInput").ap()
    xTrows = nc.dram_tensor("xTrows", [KA, R], FP, kind="ExternalInput").ap()
    wTa = nc.dram_tensor("wTa", [KA, OUT_FEAT], FP, kind="ExternalInput").ap()
    WAc = nc.dram_tensor("WAc", [KA, H], FP, kind="ExternalInput").ap()
    WAp = nc.dram_tensor("WAp", [KA, H], FP, kind="ExternalInput").ap()
    adjT = nc.dram_tensor("adjT", [N, R], BF, kind="ExternalInput").ap()
    sel8_in = nc.dram_tensor("sel8", [H, H * 128], FP, kind="ExternalInput").ap()
    outT = nc.dram_tensor("outT", [OUT_FEAT, R], FP, kind="ExternalOutput").ap()

    with tile.TileContext(nc) as tc:
        from contextlib import ExitStack
        with ExitStack() as top:
            consts = top.enter_context(tc.tile_pool(name="consts", bufs=1))
            persist = top.enter_context(tc.tile_pool(name="persist", bufs=1))

            ident = consts.tile([128, 128], FP)
            make_identity(nc, ident[:])
            ones_col = consts.tile([128, 1], BF)
            nc.vector.memset(ones_col[:], 1.0)
            sel8 = consts.tile([H, H * 128], FP)
            nc.sync.dma_start(out=sel8[:], in_=sel8_in[:])
            negone = consts.tile([128, 1], FP)
            nc.vector.memset(negone[:], -1.0)

            wk = consts.tile([128, 2, OUT_FEAT], FP)
            nc.sync.dma_start(out=wk[:, 0, :], in_=wTa[0:128, :])
            nc.sync.dma_start(out=wk[:, 1, :], in_=wTa[128:256, :])
            wk2 = consts.tile([1, OUT_FEAT], FP)
            nc.sync.dma_start(out=wk2[:], in_=wTa[256:257, :])
            wac = consts.tile([128, 2, H], FP)
            nc.sync.dma_start(out=wac[:, 0, :], in_=WAc[0:128, :])
            nc.sync.dma_start(out=wac[:, 1, :], in_=WAc[128:256, :])
            wac2 = consts.tile([1, H], FP)
            nc.sync.dma_start(out=wac2[:], in_=WAc[256:257, :])
            wap = consts.tile([128, 2, H], FP)
            nc.sync.dma_start(out=wap[:, 0, :], in_=WAp[0:128, :])
            nc.sync.dma_start(out=wap[:, 1, :], in_=WAp[128:256, :])
            wap2 = consts.tile([1, H], FP)
            nc.sync.dma_start(out=wap2[:], in_=WAp[256:257, :])

            # Persistent SBUF
            aggW = persist.tile([128, JC, H * WCOLS], BF)   # [B2NF_h | B2_h] x8
            shW = persist.tile([128, JC, OUT_FEAT + 1], BF)  # [NF | ones]
            lcn = persist.tile([128, JC, H], FP)
            b8c = persist.tile([128, JC, H], FP)             # exp(.8 lc)
            lpT = persist.tile([H, R], FP)
            a8rep = persist.tile([128, H, R], BF)            # exp(.8 lp) bcast
            a2rep = persist.tile([128, H, R], FP)            # exp(.2 lp) bcast
            scol = persist.tile([128, 2], FP)
            numT = persist.tile([128, 2, R], FP)
            outTs = persist.tile([128, 2, R], FP)
            zAll32 = persist.tile([32, R], FP)
            rzAll = persist.tile([32, R], FP)
            m3s = persist.tile([128, 2, R], FP)
            zrow6 = persist.tile([1, H, R], FP)
            degs = persist.tile([1, R], FP)
            nc.vector.memset(zAll32[:], 1.0)

            aggW_v = aggW.rearrange("p j (h w) -> p j h w", w=WCOLS)
            nc.vector.memset(shW[:, :, OUT_FEAT:OUT_FEAT + 1], 1.0)

            # ---- Phase 0: NF, lc, lp, exp factors, S ----
            with ExitStack() as ph0:
                xw = ph0.enter_context(tc.tile_pool(name="xw", bufs=3))
                ps0 = ph0.enter_context(
                    tc.tile_pool(name="ps0", bufs=4, space="PSUM"))
                for nb in range(JC):
                    xk = xw.tile([128, 2, 128], FP)
                    nc.sync.dma_start(out=xk[:, 0, :],
                                      in_=xTa[0:128, nb * 128:(nb + 1) * 128])
                    nc.sync.dma_start(out=xk[:, 1, :],
                                      in_=xTa[128:256, nb * 128:(nb + 1) * 128])
                    xk2 = xw.tile([1, 128], FP)
                    nc.sync.dma_start(out=xk2[:],
                                      in_=xTa[256:257, nb * 128:(nb + 1) * 128])
                    pnf = ps0.tile([128, OUT_FEAT], FP, space="PSUM", tag="ps0")
                    plc = ps0.tile([128, H], FP, space="PSUM", tag="ps0")
                    nc.tensor.matmul(pnf[:], xk[:, 0, :], wk[:, 0, :],
                                     start=True, stop=False)
                    nc.tensor.matmul(pnf[:], xk[:, 1, :], wk[:, 1, :],
                                     start=False, stop=False)
                    nc.tensor.matmul(pnf[:], xk2[:], wk2[:],
                                     start=False, stop=True)
                    nc.tensor.matmul(plc[:], xk[:, 0, :], wac[:, 0, :],
                                     start=True, stop=False)
                    nc.tensor.matmul(plc[:], xk[:, 1, :], wac[:, 1, :],
                                     start=False, stop=False)
                    nc.tensor.matmul(plc[:], xk2[:], wac2[:],
                                     start=False, stop=True)
                    # NF (bf16) into shared weights
                    nc.scalar.copy(shW[:, nb, 0:OUT_FEAT], pnf[:])
                    nc.scalar.copy(lcn[:, nb, :], plc[:])

                # exp factors of lc: B8 = exp(.8 lc) (scalar APs), B2 = exp(.2 lc)
                lcn_f = lcn.rearrange("p j h -> p (j h)")
                b8c_f = b8c.rearrange("p j h -> p (j h)")
                nc.scalar.activation(b8c_f, lcn_f,
                                     mybir.ActivationFunctionType.Exp,
                                     bias=0.0, scale=0.8)
                b2tmp = xw.tile([128, JC, H], FP, name="b2tmp")
                nc.scalar.activation(b2tmp.rearrange("p j h -> p (j h)"), lcn_f,
                                     mybir.ActivationFunctionType.Exp,
                                     bias=0.0, scale=0.2)
                # aggW: B2NF = NF * B2 (broadcast B2 over the 32 feature cols)
                for nb in range(JC):
                    b2b = b2tmp[:, nb, :]
                    b2bc = bass.AP(tensor=b2b.tensor, offset=b2b.offset,
                                   ap=[b2b.ap[0], b2b.ap[1], [0, D]])
                    nfv = shW[:, nb, 0:OUT_FEAT].rearrange(
                        "p (h d) -> p h d", d=D)
                    nc.gpsimd.tensor_mul(aggW_v[:, nb, :, 0:D], nfv, b2bc)
                    b2col = b2tmp[:, nb, :]
                    b2col3 = bass.AP(tensor=b2col.tensor, offset=b2col.offset,
                                     ap=[b2col.ap[0], b2col.ap[1], [0, 1]])
                    nc.gpsimd.tensor_copy(aggW_v[:, nb, :, D:D + 1], b2col3)

                # lp for this core's own rows
                for rb in range(R // 128):
                    xr = xw.tile([128, 2, 128], FP)
                    nc.sync.dma_start(out=xr[:, 0, :],
                                      in_=xTrows[0:128, rb * 128:(rb + 1) * 128])
                    nc.sync.dma_start(out=xr[:, 1, :],
                                      in_=xTrows[128:256, rb * 128:(rb + 1) * 128])
                    xr2 = xw.tile([1, 128], FP)
                    nc.sync.dma_start(
                        out=xr2[:], in_=xTrows[256:257, rb * 128:(rb + 1) * 128])
                    plp = ps0.tile([128, H], FP, space="PSUM", tag="ps0")
                    nc.tensor.matmul(plp[:], xr[:, 0, :], wap[:, 0, :],
                                     start=True, stop=False)
                    nc.tensor.matmul(plp[:], xr[:, 1, :], wap[:, 1, :],
                                     start=False, stop=False)
                    nc.tensor.matmul(plp[:], xr2[:], wap2[:],
                                     start=False, stop=True)
                    lps = xw.tile([128, H], FP)
                    nc.scalar.copy(lps[:], plp[:])
                    plpT = ps0.tile([H, 128], FP, space="PSUM", tag="ps0")
                    nc.tensor.transpose(plpT[:], lps[:], ident[:])
                    nc.scalar.copy(lpT[:, rb * 128:(rb + 1) * 128], plpT[:])

                # S = colsum(NF) from bf16 weights (matches aggregation dtype)
                psS = ps0.tile([1, OUT_FEAT], FP, space="PSUM", tag="ps0")
                for nb in range(JC):
                    nc.tensor.matmul(psS[:], ones_col[:], shW[:, nb, 0:OUT_FEAT],
                                     start=(nb == 0), stop=(nb == JC - 1))
                sS = xw.tile([1, OUT_FEAT], FP)
                nc.scalar.copy(sS[:], psS[:])
                for h in range(H):
                    pst = ps0.tile([D, 1], FP, space="PSUM", tag="ps0")
                    nc.tensor.transpose(
                        pst[:], sS[0:1, h * D:(h + 1) * D], ident[0:1, 0:1])
                    r0 = (h * D) % 128
                    nc.scalar.copy(scol[r0:r0 + D, h // 4:h // 4 + 1], pst[:])

                # A8/A2 row factors, broadcast across partitions
                a8T = xw.tile([H, R], FP)
                nc.scalar.activation(a8T[:], lpT[:],
                                     mybir.ActivationFunctionType.Exp,
                                     bias=0.0, scale=0.8)
                a2T = xw.tile([H, R], FP)
                nc.scalar.activation(a2T[:], lpT[:],
                                     mybir.ActivationFunctionType.Exp,
                                     bias=0.0, scale=0.2)
                for h in range(H):
                    pbr = ps0.tile([128, R], FP, space="PSUM", tag="psbig")
                    nc.tensor.matmul(pbr[:], sel8[:, h * 128:(h + 1) * 128],
                                     a8T[:], start=True, stop=True)
                    nc.scalar.copy(a8rep[:, h, :], pbr[:])
                    pbr2 = ps0.tile([128, R], FP, space="PSUM", tag="psbig")
                    nc.tensor.matmul(pbr2[:], sel8[:, h * 128:(h + 1) * 128],
                                     a2T[:], start=True, stop=True)
                    nc.scalar.copy(a2rep[:, h, :], pbr2[:])

            # ---- Phase 1: main j-loop, two rounds ----
            def do_round(ph, heads, with_m3, with_deg):
                acc = ph.enter_context(
                    tc.tile_pool(name="acc", bufs=1, space="PSUM"))
                stream = ph.enter_context(tc.tile_pool(name="stream", bufs=3))
                work = ph.enter_context(tc.tile_pool(name="work", bufs=3))
                pacc = {h: acc.tile([WCOLS, R], FP, space="PSUM",
                                    name=f"pacc{h}") for h in heads}
                psh = None
                pdeg = None
                if with_m3:
                    psh = [acc.tile([128, R], FP, space="PSUM", name=f"psh{k}")
                           for k in range(2)]
                if with_deg:
                    pdeg = acc.tile([1, R], FP, space="PSUM", name="pdeg")
                nh = len(heads)
                for jc in range(JC):
                    at = stream.tile([128, R], BF, name="adjT_tile")
                    nc.sync.dma_start(out=at[:],
                                      in_=adjT[jc * 128:(jc + 1) * 128, :])
                    tb = work.tile([128, nh, R], BF, name="tb")
                    s1 = work.tile([128, nh, R], BF, name="s1")
                    for k, h in enumerate(heads):
                        if A8M_ENGINE[h] == 'act':
                            # relu(a8*B8 - 1) then +1 == max(a8*B8, 1)
                            rr = work.tile([128, R], BF, name="rr")
                            nc.scalar.activation(
                                rr[:], a8rep[:, h, :],
                                mybir.ActivationFunctionType.Relu,
                                bias=negone[:], scale=b8c[:, jc, h:h + 1])
                            nc.scalar.activation(
                                tb[:, k, :], rr[:],
                                mybir.ActivationFunctionType.Copy,
                                bias=1.0, scale=1.0)
                        else:
                            nc.vector.tensor_scalar(
                                tb[:, k, :], a8rep[:, h, :],
                                b8c[:, jc, h:h + 1], 1.0,
                                mybir.AluOpType.mult, mybir.AluOpType.max)
                    # mask multiply: fused across heads per engine
                    dve_ks = [k for k, h in enumerate(heads)
                              if TT_ENGINE[h] == 'dve']
                    gp_ks = [k for k, h in enumerate(heads)
                             if TT_ENGINE[h] == 'gp']
                    for eng, ks in ((nc.vector, dve_ks), (nc.gpsimd, gp_ks)):
                        for k0, k1 in _runs(ks):
                            cnt = k1 - k0
                            atb = bass.AP(tensor=at.tensor, offset=at.offset,
                                          ap=[at.ap[0], [0, cnt], at.ap[1]])
                            eng.tensor_mul(s1[:, k0:k1, :], tb[:, k0:k1, :],
                                           atb)
                    for k, h in enumerate(heads):
                        nc.tensor.matmul(
                            pacc[h][:],
                            aggW[:, jc, h * WCOLS:(h + 1) * WCOLS],
                            s1[:, k, :],
                            start=(jc == 0), stop=(jc == JC - 1))
                    if with_m3:
                        nc.tensor.matmul(psh[0][:], shW[:, jc, 0:128], at[:],
                                         start=(jc == 0), stop=(jc == JC - 1))
                        nc.tensor.matmul(psh[1][:], shW[:, jc, 128:256], at[:],
                                         start=(jc == 0), stop=(jc == JC - 1))
                    if with_deg:
                        nc.tensor.matmul(
                            pdeg[:], shW[:, jc, OUT_FEAT:OUT_FEAT + 1], at[:],
                            start=(jc == 0), stop=(jc == JC - 1))
                return pacc, psh, pdeg, work

            with ExitStack() as ph1:
                paccA, psh, _, workA = do_round(ph1, ROUND_A, True, False)
                for h in ROUND_A:
                    _head_epilogue(nc, h, paccA[h], psh[h // 4][
                        (h * D) % 128:(h * D) % 128 + D, :], a2rep, scol,
                        numT, zrow6, workA)
                # save M3 to SBUF before psum banks are released
                nc.scalar.copy(m3s[:, 0, :], psh[0][:])
                nc.scalar.copy(m3s[:, 1, :], psh[1][:])
            with ExitStack() as ph1b:
                paccB, _, pdeg, workB = do_round(ph1b, ROUND_B, False, True)
                for h in ROUND_B:
                    _head_epilogue(nc, h, paccB[h], m3s[
                        (h * D) % 128:(h * D) % 128 + D, h // 4, :], a2rep,
                        scol, numT, zrow6, workB)
                nc.scalar.copy(degs[:], pdeg[:])
                # finalize Z rows: z = zpart + N - deg, packed to zAll32
                for h in range(H):
                    ztmp = workB.tile([1, R], FP, name="ztmp")
                    nc.vector.scalar_tensor_tensor(
                        ztmp[:], zrow6[0:1, h, :], float(N), degs[:],
                        mybir.AluOpType.add, mybir.AluOpType.subtract)
                    nc.sync.dma_start(out=zAll32[h:h + 1, :], in_=ztmp[:])

            nc.vector.reciprocal(rzAll[:], zAll32[:])

            with ExitStack() as ph2:
                ps2 = ph2.enter_context(
                    tc.tile_pool(name="ps2", bufs=2, space="PSUM"))
                for h in range(H):
                    r0 = (h * D) % 128
                    ch = h // 4
                    pz = ps2.tile([D, R], FP, space="PSUM")
                    nc.tensor.matmul(pz[:], sel8[:, h * 128:h * 128 + D],
                                     rzAll[0:H, :], start=True, stop=True)
                    nc.vector.tensor_mul(
                        outTs[r0:r0 + D, ch, :], numT[r0:r0 + D, ch, :], pz[:])
                nc.sync.dma_start(out=outT[0:128, :], in_=outTs[:, 0, :])
                nc.sync.dma_start(out=outT[128:256, :], in_=outTs[:, 1, :])

    nc.compile()
    return nc


def _runs(ks):
    """Contiguous runs [k0, k1) in a sorted index list."""
    out = []
    for k in ks:
        if out and out[-1][1] == k:
            out[-1][1] = k + 1
        else:
            out.append([k, k + 1])
    return [tuple(x) for x in out]


def _head_epilogue(nc, h, pacc, m3, a2rep, scol, numT, zrow6, work):
    """numT_h = S[c] + A2[i]*(u'@B2NF) - M3[c,i]; zrow6_h = A2*(u'@B2)."""
    r0 = (h * D) % 128
    ch = h // 4
    t1 = work.tile([128, R], mybir.dt.float32, name="t1")
    nc.vector.tensor_mul(t1[r0:r0 + D, :], pacc[0:D, :],
                         a2rep[r0:r0 + D, h, :])
    nc.vector.scalar_tensor_tensor(
        numT[r0:r0 + D, ch, :], t1[r0:r0 + D, :],
        scol[r0:r0 + D, ch:ch + 1], m3,
        mybir.AluOpType.add, mybir.AluOpType.subtract)
    nc.vector.tensor_mul(zrow6[0:1, h, :], pacc[D:D + 1, :],
                         a2rep[32:33, h, :])


_PROGRAM_CACHE = {}


def kernel(x, W, b, a, adj_matrix):
    x = np.asarray(x, dtype=np.float32)
    W = np.asarray(W, dtype=np.float32)
    b = np.asarray(b, dtype=np.float32)
    a = np.asarray(a, dtype=np.float32)
    adj = np.asarray(adj_matrix, dtype=np.float32)

    xTa = np.ascontiguousarray(
        np.vstack([x.T, np.ones((1, N), np.float32)]))            # [257, N]
    wTa = np.ascontiguousarray(np.vstack([W.T, b[None, :]]))      # [257, 256]
    Ap = np.zeros((OUT_FEAT, H), np.float32)
    Ac = np.zeros((OUT_FEAT, H), np.float32)
    for h in range(H):
        Ap[h * D:(h + 1) * D, h] = a[h, :D]
        Ac[h * D:(h + 1) * D, h] = a[h, D:]
    WAp = np.ascontiguousarray(wTa @ Ap)
    WAc = np.ascontiguousarray(wTa @ Ac)

    sel8_host = np.zeros((H, H * 128), np.float32)
    for h in range(H):
        sel8_host[h, h * 128:(h + 1) * 128] = 1.0

    if "nc" not in _PROGRAM_CACHE:
        _PROGRAM_CACHE["nc"] = build_program()
    nc = _PROGRAM_CACHE["nc"]

    in_maps = []
    for c in range(N_CORES):
        rows = slice(c * R, (c + 1) * R)
        in_maps.append({
            "xTa": xTa,
            "xTrows": np.ascontiguousarray(xTa[:, rows]),
            "wTa": wTa,
            "WAc": WAc,
            "WAp": WAp,
            "adjT": np.ascontiguousarray(adj[rows, :].T).astype(
                ml_dtypes.bfloat16),
            "sel8": sel8_host,
        })

    res = run_bass_kernel_spmd(nc, in_maps, list(range(N_CORES)))
    out = np.empty((N, OUT_FEAT), np.float32)
    for c in range(N_CORES):
        out[c * R:(c + 1) * R, :] = res.results[c]["outT"].T
    return out
